# revision 31
# baseline (speedup 1.0000x reference)
"""Causal self-attention (B=2, T=4096, C=768, H=12) on 8 TRN2 NeuronCores.

Sharding: core c -> batch c//4, heads 3*(c%4) .. 3*(c%4)+2.  Each core is
fully independent (no collectives): it computes qkv for its 3 heads from
x[b], runs causal flash attention, and produces the partial output
projection outT = (Y_heads @ W_proj[rows]).T of shape [C, T].  The host
sums the 4 per-batch partials, transposes, and adds b_proj.

Engine-balanced design (ACT exp ~235us is the floor; everything else
hides under it):
  - PE: S^T pairs in disjoint row-groups (base-0/base-64) AND PV split
    into two 64-contraction halves (rows 0-63 -> ytA bank, 64-127 -> ytB
    bank) that also run concurrently.  Head-major loop so only 2 yt PSUM
    banks live; next-group QKV and prev-group proj interleave as filler.
  - ACT: only the 216 exp activations.
  - DVE: qkT bias-adds, vnat drains (+v-bias fold), ytA+ytB merge,
    reciprocal, normalize-mul, proj drains.
  - Pool: causal masks via in-place affine_select on p2, reciprocal
    partition-broadcast.
  - DMA: half-swapped qkT duplicates via SBUF->SBUF copies.
"""

import os
import sys

import numpy as np

for _p in ("/opt/trn_rl_repo", "/root/.axon_site/_ro/trn_rl_repo"):
    if os.path.isdir(_p) and _p not in sys.path:
        sys.path.insert(0, _p)

from contextlib import ExitStack

import concourse.bacc as bacc
import concourse.bass as bass
import concourse.mybir as mybir
import concourse.tile as tile
from concourse.bass_utils import run_bass_kernel_spmd

F32 = mybir.dt.float32
F32R = mybir.dt.float32r
EXP = mybir.ActivationFunctionType.Exp
IS_GE = mybir.AluOpType.is_ge

B, T_FULL, C = 2, 4096, 768
H, DH = 12, 64
HPC = 3                      # heads per core
NCORES = 8
P = 128
QG = 512                     # query-group span (free dim of S^T tiles)
KT = 128                     # key tile (partition dim of S^T tiles)
NQK = 6 * DH                 # 384 rows of qkT (q,k for 3 heads)
NVP = 256                    # padded v matmul width: v0|1|v1|1|v2|1|zeros
VW = 3 * 65                  # = 195 columns of vnat actually kept
SCALE = 1.0 / np.sqrt(DH)

# qkT row layout as (block, local_head) pairs of 64 rows each
QK_ORDER = [("q", 0), ("q", 1), ("k", 0), ("k", 1), ("q", 2), ("k", 2)]
BLK = {"q": 0, "k": 1, "v": 2}


def r32(ap):
    return ap.bitcast(F32R)


def build_nc(t=T_FULL, debug=False):
    ng = t // QG             # query groups
    nc = bacc.Bacc(None, target_bir_lowering=False)
    dbg = {}
    if debug:
        for name, shape in [("d_qkt0", [P, t]), ("d_qkd0", [P, t]),
                            ("d_vn0", [P, VW]), ("d_bvt", [P, NVP]),
                            ("d_p2", [P, 2 * QG]), ("d_rb", [DH, QG])]:
            dbg[name] = nc.declare_dram_parameter(name, shape, F32,
                                                  isOutput=True)
        nc._dbg_tensors = list(dbg)
    xT = nc.declare_dram_parameter("xT", [C, t], F32, isOutput=False)
    wqk = nc.declare_dram_parameter("wqk", [C, NQK], F32, isOutput=False)
    bqk = nc.declare_dram_parameter("bqk", [NQK, 1], F32, isOutput=False)
    wvp = nc.declare_dram_parameter("wvp", [C, NVP], F32, isOutput=False)
    bvrow = nc.declare_dram_parameter("bvrow", [1, NVP], F32, isOutput=False)
    wp = nc.declare_dram_parameter("wp", [HPC * DH, C], F32, isOutput=False)
    outT = nc.declare_dram_parameter("outT", [C, t], F32, isOutput=True)

    with tile.TileContext(nc) as tc, ExitStack() as ctx:
        const = ctx.enter_context(tc.tile_pool(name="const", bufs=1))
        qkp = ctx.enter_context(tc.tile_pool(name="qk", bufs=1))
        vp = ctx.enter_context(tc.tile_pool(name="vn", bufs=1))
        xpool = ctx.enter_context(tc.tile_pool(name="xin", bufs=2))
        ppool = ctx.enter_context(tc.tile_pool(name="pp", bufs=3))
        ytsb = ctx.enter_context(tc.tile_pool(name="ytsb", bufs=2))
        ytssb = ctx.enter_context(tc.tile_pool(name="ytssb", bufs=2))
        rrp = ctx.enter_context(tc.tile_pool(name="rr", bufs=2))
        osb = ctx.enter_context(tc.tile_pool(name="osb", bufs=2))
        spsum = ctx.enter_context(tc.tile_pool(name="sps", bufs=2, space="PSUM"))
        ytps = ctx.enter_context(tc.tile_pool(name="ytps", bufs=1, space="PSUM"))
        aux = ctx.enter_context(tc.tile_pool(name="aux", bufs=1, space="PSUM"))
        projp = ctx.enter_context(tc.tile_pool(name="projp", bufs=1, space="PSUM"))

        # ---- constants -------------------------------------------------
        # DMA order is the critical-path order: the k-th qk matmul of the
        # prologue only needs wqk[k] and x0[k], so interleave them and
        # defer everything not needed until later in the stream.
        wqk_sb, wvp_sb, x0_sb = [], [], []
        b_sb = []
        bvr_sb = const.tile([1, NVP], F32, tag="bvr", name="bvr")
        for k in range(6):
            tl = const.tile([P, NQK], F32R, tag=f"wqk{k}", name=f"wqk{k}")
            nc.sync.dma_start(tl[:], r32(wqk[k * P:(k + 1) * P, :]))
            wqk_sb.append(tl)
            tx = xpool.tile([P, QG], F32R, tag=f"x{k}", name=f"x{k}")
            nc.sync.dma_start(tx[:], r32(xT[k * P:(k + 1) * P, 0:QG]))
            x0_sb.append(tx)
            if k == 0:
                for m in range(3):
                    tb = const.tile([P, 1], F32, tag=f"bq{m}", name=f"bq{m}")
                    nc.sync.dma_start(tb[:], bqk[m * P:(m + 1) * P, :])
                    b_sb.append(tb)
                nc.sync.dma_start(bvr_sb[:], bvrow[:, :])
        for k in range(6):
            tv = const.tile([P, NVP], F32R, tag=f"wvp{k}", name=f"wvp{k}")
            nc.sync.dma_start(tv[:], r32(wvp[k * P:(k + 1) * P, :]))
            wvp_sb.append(tv)
        # v-bias row (with 1.0 in the ones columns), broadcast to all
        # partitions once: vnat drain adds it, so PV emits (Y + bv*denom)
        # rows plus the raw denominator, and the normalize-divide yields
        # Y/denom + bv with no separate bias op.
        bv_tile = const.tile([P, NVP], F32, tag="bvt", name="bvt")
        nc.gpsimd.partition_broadcast(bv_tile[:], bvr_sb[:])
        # wp1 rows are duplicated at partitions 0-63 AND 64-127 so the
        # wp1 matmuls of adjacent proj column-tiles can run concurrently
        # in disjoint PE row-groups.
        wp0 = const.tile([P, C], F32R, tag="wp0", name="wp0")
        wp1d = const.tile([P, C], F32R, tag="wp1d", name="wp1d")
        nc.sync.dma_start(wp0[:], r32(wp[0:P, :]))
        nc.sync.dma_start(wp1d[0:DH, :], r32(wp[P:P + DH, :]))
        nc.sync.dma_start(wp1d[DH:P, :], r32(wp[P:P + DH, :]))

        # ---- persistent qkT / v storage --------------------------------
        # qkT row layout: A=[q0|q1] B=[k0|k1] C=[q2|k2], plus half-swapped
        # duplicates A'=[q1|q0] B'=[k1|k0] C'=[k2|q2] so every head has its
        # (q, k) pair available at base partition 0 AND base partition 64.
        qkt = [qkp.tile([P, t], F32R, tag=f"qkt{i}", name=f"qkt{i}")
               for i in range(3)]          # A, B, C
        qkd = [qkp.tile([P, t], F32R, tag=f"qkd{i}", name=f"qkd{i}")
               for i in range(3)]          # A', B', C'
        A, Bt, Ct = qkt
        Ad, Bd, Cd = qkd
        qk0 = [(A, Bt), (Ad, Bd), (Ct, Cd)]      # base-0 (q_tile, k_tile)
        qk64 = [(Ad, Bd), (A, Bt), (Cd, Ct)]     # base-64 (q_tile, k_tile)
        # v in natural orientation: per 128-row t-tile, cols h*65..h*65+64
        # hold [v_h | ones]
        vnat = [vp.tile([P, VW], F32R, tag=f"vn{j}", name=f"vn{j}")
                for j in range(t // P)]

        def v_ap(h, j):
            return vnat[j][:, 65 * h:65 * h + DH + 1]

        # ---- per-group QKV emission (used as PE filler) ----------------
        def gen_qkv(g):
            """Step-granular generator for group g's QKV filler: one PE
            matmul per step, so the weave can spread them between
            attention matmuls (keeps the PE cadence smooth and avoids
            back-to-back accumulates into the same PSUM bank)."""
            gs = slice(g * QG, (g + 1) * QG)
            if g == 0:
                xtiles[0] = x0_sb           # preloaded with the constants
            else:
                xk = []
                for k in range(6):
                    tl = xpool.tile([P, QG], F32R, tag=f"x{k}", name=f"x{k}")
                    nc.sync.dma_start(tl[:], r32(xT[k * P:(k + 1) * P, gs]))
                    xk.append(tl)
                xtiles[g % 2] = xk
            yield
            xk = xtiles[g % 2]
            for m in range(3):              # qkT m-tiles (A, B, C)
                ps = aux.tile([P, QG], F32, tag="aux", name="qkps")
                for k in range(6):
                    nc.tensor.matmul(ps[:], wqk_sb[k][:, m * P:(m + 1) * P],
                                     xk[k][:], start=(k == 0), stop=(k == 5))
                    if k < 5:
                        yield
                nc.vector.tensor_scalar_add(qkt[m][:, gs], ps[:], b_sb[m][:])
                # half-swapped duplicate via SBUF->SBUF DMA (frees ACT)
                nc.sync.dma_start(qkd[m][DH:, gs], qkt[m][:DH, gs])
                nc.sync.dma_start(qkd[m][:DH, gs], qkt[m][DH:, gs])
                yield
            for ti in range(4):             # v t-tiles
                j = 4 * g + ti
                ps = aux.tile([P, NVP], F32, tag="aux", name="vnps")
                for k in range(6):
                    nc.tensor.matmul(ps[:], xk[k][:, ti * P:(ti + 1) * P],
                                     wvp_sb[k][:], start=(k == 0), stop=(k == 5))
                    if k < 5:
                        yield
                nc.vector.tensor_add(vnat[j][:], ps[:, :VW], bv_tile[:, :VW])
                yield

        def gen_proj(g, y0, y1d):
            """Step-granular projection for group g.  wp1 matmuls of
            adjacent column-tiles pair up in disjoint PE row-groups
            (even cm: rows 0-63 vs y1d[0:64]; odd cm: rows 64-127 vs
            y1d[64:128])."""
            gq = slice(g * QG, (g + 1) * QG)
            for cme in range(0, 6, 2):
                opA = projp.tile([P, QG], F32, tag="opA", name="ooA")
                nc.tensor.matmul(opA[:], wp0[:, cme * P:(cme + 1) * P],
                                 y0[:], start=True, stop=False)
                yield
                opB = projp.tile([P, QG], F32, tag="opB", name="ooB")
                nc.tensor.matmul(opB[:], wp0[:, (cme + 1) * P:(cme + 2) * P],
                                 y0[:], start=True, stop=False)
                yield
                nc.tensor.matmul(opA[:], wp1d[0:DH, cme * P:(cme + 1) * P],
                                 y1d[0:DH, :], start=False, stop=True)
                nc.tensor.matmul(opB[:], wp1d[DH:P, (cme + 1) * P:(cme + 2) * P],
                                 y1d[DH:P, :], start=False, stop=True)
                for cm, op in ((cme, opA), (cme + 1, opB)):
                    ob = osb.tile([P, QG], F32, tag="ob", name="ob")
                    nc.vector.tensor_copy(ob[:], op[:])
                    nc.sync.dma_start(outT[cm * P:(cm + 1) * P, gq], ob[:])
                yield

        def emit_S(g, h, pr, npairs):
            """S^T pair + exp (+mask); returns the pending-PV job."""
            gs = slice(g * QG, (g + 1) * QG)
            q0t, k0t = qk0[h]
            q64t, k64t = qk64[h]
            j0, j1 = 2 * pr, 2 * pr + 1
            s2 = spsum.tile([P, 2 * QG], F32, tag="s", name="s")
            # two k-tiles in disjoint PE row-groups (base 0 / base 64) ->
            # the array runs them concurrently
            nc.tensor.matmul(
                s2[:, :QG],
                k0t[0:DH, j0 * P:(j0 + 1) * P],
                q0t[0:DH, gs],
                start=True, stop=True,
            )
            nc.tensor.matmul(
                s2[:, QG:],
                k64t[DH:P, j1 * P:(j1 + 1) * P],
                q64t[DH:P, gs],
                start=True, stop=True,
            )
            p2 = ppool.tile([P, 2 * QG], F32R, tag="p", name="p")
            nc.scalar.activation(p2[:], s2[:], EXP, scale=float(SCALE))
            if pr >= npairs - 2:
                # causal mask for the 2 diagonal k-tiles, in-place on the
                # Pool engine: keep col q of row k of half j iff
                # (g*512 + q) - (j*128 + k) >= 0.
                nc.gpsimd.affine_select(
                    out=p2[:],
                    in_=p2[:],
                    compare_op=IS_GE,
                    fill=0.0,
                    base=g * QG - j0 * KT,
                    pattern=[[-KT, 2], [1, QG]],
                    channel_multiplier=-1,
                )
            if debug and g == 0 and h == 0 and pr == npairs - 1:
                nc.sync.dma_start(dbg["d_p2"][:, :], p2[:].bitcast(F32))
            return (g, h, pr, npairs, p2)

        cur_yt = [None]          # live yt PSUM accumulator

        def emit_PV_half(job, half):
            """One PV matmul (k-tile half of a pending job)."""
            g, h, pr, npairs, p2 = job
            first = (pr == 0) and (half == 0)
            last = (pr == npairs - 1) and (half == 1)
            j = 2 * pr + half
            if first:
                cur_yt[0] = ytps.tile([DH + 1, QG], F32, tag="yt",
                                      name="yt")
            yt = cur_yt[0]
            nc.tensor.matmul(yt[:], v_ap(h, j),
                             p2[:, half * QG:(half + 1) * QG],
                             start=first, stop=last)

        def emit_norm(job, yt0, y1d):
            """Head-end normalization: divide rows 0-63 by the
            denominator row (row 64), all off-PE."""
            g, h, pr, npairs, p2 = job
            yt = cur_yt[0]
            # reciprocal_approx_fast requires a partition-base-0 SBUF
            # input on hardware: bounce the denominator row first.
            d_f = rrp.tile([1, QG], F32, tag="df", name="df")
            nc.vector.tensor_copy(d_f[:], yt[DH:DH + 1, :])
            r_f = rrp.tile([1, QG], F32, tag="rf", name="rf")
            nc.vector.reciprocal_approx_fast(r_f[:], d_f[:])
            Rb = rrp.tile([DH, QG], F32, tag="Rb", name="Rb")
            nc.gpsimd.partition_broadcast(Rb[:], r_f[:])
            if h < 2:
                nc.vector.tensor_mul(yt0[DH * h:DH * (h + 1), :],
                                     yt[:DH, :], Rb[:])
            else:
                # head 2 lands in BOTH halves of y1d for the paired
                # wp1 projection matmuls
                nc.vector.tensor_mul(y1d[0:DH, :], yt[:DH, :], Rb[:])
                nc.vector.tensor_mul(y1d[DH:P, :], yt[:DH, :], Rb[:])
            if debug and g == 0 and h == 0:
                nc.sync.dma_start(dbg["d_rb"][:, :], Rb[:])

        xtiles = [None, None]
        # prologue: group 0's QKV back-to-back
        for _ in gen_qkv(0):
            pass

        # ---- fused attention, software-pipelined: PV trails S by DEPTH
        # ---- pair-slots (continuously across heads and groups) so the
        # ---- exp+mask latency never stalls the PE.  QKV of group g+1
        # ---- and proj of group g-1 weave in as single-matmul filler
        # ---- steps between the attention matmuls.
        DEPTH = 2
        # filler-step counts per group (for ratio pacing)
        NF_QKV = 1 + 3 * 6 + 4 * 6       # steps yielded by gen_qkv
        NF_PROJ = 9                      # steps yielded by gen_proj
        pend = []                # jobs with S emitted, PV outstanding
        yts_of = {}              # group -> (yt0, y1d)
        prev_y = None
        for g in range(ng):
            npairs = 2 * (g + 1)
            yt0 = ytsb.tile([P, QG], F32R, tag="yt0", name="yt0")
            y1d = ytsb.tile([P, QG], F32R, tag="y1d", name="y1d")
            yts_of[g] = (yt0, y1d)
            # filler: next group's QKV interleaved with prev group's proj
            fill = []            # list of (kind, generator)
            nf_total = 0
            if g + 1 < ng:
                fill.append(("qkv", gen_qkv(g + 1)))
                nf_total += NF_QKV
            if prev_y is not None:
                fill.append(("proj", gen_proj(g - 1, *prev_y)))
                nf_total += NF_PROJ

            def pull_step(g=g):
                """Advance the filler by one step; False when dry or
                blocked (proj before the prev group's norm is out)."""
                while fill:
                    kind, it = fill[0]
                    if kind == "proj" and any(j[0] < g for j in pend):
                        # rotate: try a later generator if present
                        if len(fill) > 1:
                            fill.append(fill.pop(0))
                            continue
                        return False
                    try:
                        next(it)
                        return True
                    except StopIteration:
                        fill.pop(0)
                return False

            n_att = npairs * HPC
            next_fill = 0
            ui = 0
            for h in range(HPC):
                for pr in range(npairs):
                    pend.append(emit_S(g, h, pr, npairs))
                    due = ui * nf_total // n_att - next_fill
                    jb = pend.pop(0) if len(pend) > DEPTH else None
                    if jb is not None:
                        if due > 0 and pull_step():
                            next_fill += 1
                            due -= 1
                        emit_PV_half(jb, 0)
                        if due > 0 and pull_step():
                            next_fill += 1
                            due -= 1
                        emit_PV_half(jb, 1)
                        if jb[2] == jb[3] - 1:          # pr == npairs-1
                            emit_norm(jb, *yts_of[jb[0]])
                    while due > 0 and pull_step():
                        next_fill += 1
                        due -= 1
                    ui += 1
            while pull_step():
                pass
            prev_y = (yt0, y1d)
        # epilogue: drain the pipeline, then the last group's projection
        while pend:
            jb = pend.pop(0)
            emit_PV_half(jb, 0)
            emit_PV_half(jb, 1)
            if jb[2] == jb[3] - 1:
                emit_norm(jb, *yts_of[jb[0]])
        for _ in gen_proj(ng - 1, *prev_y):
            pass
        if debug:
            nc.sync.dma_start(dbg["d_qkt0"][:, :], qkt[0][:].bitcast(F32))
            nc.sync.dma_start(dbg["d_qkd0"][:, :], qkd[0][:].bitcast(F32))
            nc.sync.dma_start(dbg["d_vn0"][:, :], vnat[0][:].bitcast(F32))
            nc.sync.dma_start(dbg["d_bvt"][:, :], bv_tile[:])
    nc.compile()
    return nc


_NC_CACHE = {}


def get_nc(t=T_FULL):
    if t not in _NC_CACHE:
        _NC_CACHE[t] = build_nc(t)
    return _NC_CACHE[t]


def make_in_maps(x, W_attn, b_attn, W_proj):
    x = np.ascontiguousarray(np.asarray(x, np.float32))
    W_attn = np.asarray(W_attn, np.float32)
    b_attn = np.asarray(b_attn, np.float32)
    W_proj = np.asarray(W_proj, np.float32)
    in_maps = []
    for c in range(NCORES):
        b = c // 4
        hs = [3 * (c % 4) + i for i in range(HPC)]
        cols = [W_attn[:, BLK[kind] * C + hs[lh] * DH:
                       BLK[kind] * C + (hs[lh] + 1) * DH]
                for kind, lh in QK_ORDER]
        wqk = np.ascontiguousarray(np.concatenate(cols, axis=1))
        bqk = np.concatenate(
            [b_attn[BLK[kind] * C + hs[lh] * DH:BLK[kind] * C + (hs[lh] + 1) * DH]
             for kind, lh in QK_ORDER]
        ).reshape(NQK, 1)
        wvp = np.zeros((C, NVP), np.float32)
        bvrow = np.zeros((1, NVP), np.float32)
        for lh in range(HPC):
            wvp[:, 65 * lh:65 * lh + DH] = \
                W_attn[:, 2 * C + hs[lh] * DH:2 * C + (hs[lh] + 1) * DH]
            bvrow[0, 65 * lh:65 * lh + DH] = \
                b_attn[2 * C + hs[lh] * DH:2 * C + (hs[lh] + 1) * DH]
            bvrow[0, 65 * lh + DH] = 1.0
        wp = np.ascontiguousarray(
            np.concatenate([W_proj[h * DH:(h + 1) * DH, :] for h in hs], axis=0)
        )
        xTc = np.ascontiguousarray(x[b].T)
        in_maps.append({
            "xT": xTc,
            "wqk": wqk,
            "bqk": np.ascontiguousarray(bqk),
            "wvp": wvp,
            "bvrow": bvrow,
            "wp": wp,
        })
    return in_maps


def unshard(per_core_outT, b_proj):
    t = per_core_outT[0].shape[1]
    out = np.zeros((B, t, C), np.float32)
    for c in range(NCORES):
        out[c // 4] += per_core_outT[c].T
    out += np.asarray(b_proj, np.float32)[None, None, :]
    return out


def kernel(x, W_attn, b_attn, W_proj, b_proj, **run_kwargs):
    nc = get_nc(T_FULL)
    in_maps = make_in_maps(x, W_attn, b_attn, W_proj)
    res = None
    last_err = None
    for attempt in range(3):
        try:
            res = run_bass_kernel_spmd(nc, in_maps,
                                       core_ids=list(range(NCORES)),
                                       **run_kwargs)
            break
        except Exception as e:  # transient NRT_EXEC_UNIT_UNRECOVERABLE etc.
            last_err = e
    if res is None:
        raise last_err
    outs = [res.results[c]["outT"] for c in range(NCORES)]
    out = unshard(outs, b_proj)
    return out


# revision 33
# speedup vs baseline: 1.2273x; 1.2273x over previous
"""Causal self-attention (B=2, T=4096, C=768, H=12) on 8 TRN2 NeuronCores.

Sharding: core c -> batch c//4, heads 3*(c%4) .. 3*(c%4)+2.  Each core is
fully independent (no collectives): it computes qkv for its 3 heads from
x[b], runs causal flash attention, and produces the partial output
projection outT = (Y_heads @ W_proj[rows]).T of shape [C, T].  The host
sums the 4 per-batch partials, transposes, and adds b_proj.

Engine-balanced design (ACT exp ~230us is the floor; everything else
hides under it):
  - bf16 datapath everywhere outside PSUM: halves SBUF bandwidth (the
    PE streams compete with ACT/DVE/Pool/DMA for SBUF ports) and
    halves the x/weight DMA; matmuls run 1 cyc/row in bf16.
  - PE: S^T pairs in disjoint row-groups (base-0/base-64); PV trails S
    by DEPTH pair-slots (software pipeline, continuous across heads and
    groups) so the exp latency never stalls the PE; QKV of group g+1
    and proj of group g-1 weave in as single-matmul filler steps.
  - ACT: only the 216 exp activations.
  - DVE: qkT bias-adds, vnat drains (+v-bias fold), reciprocal,
    normalize-mul, proj drains.
  - Pool: causal masks via in-place affine_select on p2, reciprocal
    partition-broadcast.
  - DMA: half-swapped qkT duplicates via SBUF->SBUF copies.
"""

import os
import sys

import numpy as np

for _p in ("/opt/trn_rl_repo", "/root/.axon_site/_ro/trn_rl_repo"):
    if os.path.isdir(_p) and _p not in sys.path:
        sys.path.insert(0, _p)

from contextlib import ExitStack

import concourse.bacc as bacc
import concourse.bass as bass
import concourse.mybir as mybir
import concourse.tile as tile
from concourse.bass_utils import run_bass_kernel_spmd

F32 = mybir.dt.float32
BF16 = mybir.dt.bfloat16
EXP = mybir.ActivationFunctionType.Exp
IS_GE = mybir.AluOpType.is_ge

B, T_FULL, C = 2, 4096, 768
H, DH = 12, 64
HPC = 3                      # heads per core
NCORES = 8
P = 128
QG = 512                     # query-group span (free dim of S^T tiles)
KT = 128                     # key tile (partition dim of S^T tiles)
NQK = 6 * DH                 # 384 rows of qkT (q,k for 3 heads)
NVP = 256                    # padded v matmul width: v0|1|v1|1|v2|1|zeros
VW = 3 * 65                  # = 195 columns of vnat actually kept
SCALE = 1.0 / np.sqrt(DH)

# qkT row layout as (block, local_head) pairs of 64 rows each
QK_ORDER = [("q", 0), ("q", 1), ("k", 0), ("k", 1), ("q", 2), ("k", 2)]
BLK = {"q": 0, "k": 1, "v": 2}


def build_nc(t=T_FULL, debug=False):
    ng = t // QG             # query groups
    nc = bacc.Bacc(None, target_bir_lowering=False)
    dbg = {}
    if debug:
        for name, shape, dt in [("d_qkt0", [P, t], BF16),
                                ("d_qkd0", [P, t], BF16),
                                ("d_vn0", [P, VW], BF16),
                                ("d_bvt", [P, NVP], F32),
                                ("d_p2", [P, 2 * QG], BF16),
                                ("d_rb", [DH, QG], F32)]:
            dbg[name] = nc.declare_dram_parameter(name, shape, dt,
                                                  isOutput=True)
        nc._dbg_tensors = list(dbg)
    xT = nc.declare_dram_parameter("xT", [C, t], BF16, isOutput=False)
    wqk = nc.declare_dram_parameter("wqk", [C, NQK], BF16, isOutput=False)
    bqk = nc.declare_dram_parameter("bqk", [NQK, 1], F32, isOutput=False)
    wvp = nc.declare_dram_parameter("wvp", [C, NVP], BF16, isOutput=False)
    bvrow = nc.declare_dram_parameter("bvrow", [1, NVP], F32, isOutput=False)
    wp = nc.declare_dram_parameter("wp", [HPC * DH, C], BF16, isOutput=False)
    outT = nc.declare_dram_parameter("outT", [C, t], F32, isOutput=True)

    with tile.TileContext(nc) as tc, ExitStack() as ctx:
        const = ctx.enter_context(tc.tile_pool(name="const", bufs=1))
        qkp = ctx.enter_context(tc.tile_pool(name="qk", bufs=1))
        vp = ctx.enter_context(tc.tile_pool(name="vn", bufs=1))
        xpool = ctx.enter_context(tc.tile_pool(name="xin", bufs=2))
        ppool = ctx.enter_context(tc.tile_pool(name="pp", bufs=4))
        ytsb = ctx.enter_context(tc.tile_pool(name="ytsb", bufs=2))
        rrp = ctx.enter_context(tc.tile_pool(name="rr", bufs=2))
        osb = ctx.enter_context(tc.tile_pool(name="osb", bufs=2))
        spsum = ctx.enter_context(tc.tile_pool(name="sps", bufs=2, space="PSUM"))
        ytps = ctx.enter_context(tc.tile_pool(name="ytps", bufs=2, space="PSUM"))
        aux = ctx.enter_context(tc.tile_pool(name="aux", bufs=2, space="PSUM"))

        # ---- constants -------------------------------------------------
        # DMA order is the critical-path order: the k-th qk matmul of the
        # prologue only needs wqk[k] and x0[k], so interleave them and
        # defer everything not needed until later in the stream.
        wqk_sb, wvp_sb, x0_sb = [], [], []
        b_sb = []
        bvr_sb = const.tile([1, NVP], F32, tag="bvr", name="bvr")
        for k in range(6):
            tl = const.tile([P, NQK], BF16, tag=f"wqk{k}", name=f"wqk{k}")
            nc.sync.dma_start(tl[:], wqk[k * P:(k + 1) * P, :])
            wqk_sb.append(tl)
            tx = xpool.tile([P, QG], BF16, tag=f"x{k}", name=f"x{k}")
            nc.sync.dma_start(tx[:], xT[k * P:(k + 1) * P, 0:QG])
            x0_sb.append(tx)
            if k == 0:
                for m in range(3):
                    tb = const.tile([P, 1], F32, tag=f"bq{m}", name=f"bq{m}")
                    nc.sync.dma_start(tb[:], bqk[m * P:(m + 1) * P, :])
                    b_sb.append(tb)
                nc.sync.dma_start(bvr_sb[:], bvrow[:, :])
        for k in range(6):
            tv = const.tile([P, NVP], BF16, tag=f"wvp{k}", name=f"wvp{k}")
            nc.sync.dma_start(tv[:], wvp[k * P:(k + 1) * P, :])
            wvp_sb.append(tv)
        # v-bias row (with 1.0 in the ones columns), broadcast to all
        # partitions once: vnat drain adds it, so PV emits (Y + bv*denom)
        # rows plus the raw denominator, and the normalize-divide yields
        # Y/denom + bv with no separate bias op.
        bv_tile = const.tile([P, NVP], F32, tag="bvt", name="bvt")
        nc.gpsimd.partition_broadcast(bv_tile[:], bvr_sb[:])
        wp0 = const.tile([P, C], BF16, tag="wp0", name="wp0")
        wp1 = const.tile([DH, C], BF16, tag="wp1", name="wp1")
        nc.sync.dma_start(wp0[:], wp[0:P, :])
        nc.sync.dma_start(wp1[:], wp[P:P + DH, :])

        # ---- persistent qkT / v storage --------------------------------
        # qkT row layout: A=[q0|q1] B=[k0|k1] C=[q2|k2], plus half-swapped
        # duplicates A'=[q1|q0] B'=[k1|k0] C'=[k2|q2] so every head has its
        # (q, k) pair available at base partition 0 AND base partition 64.
        qkt = [qkp.tile([P, t], BF16, tag=f"qkt{i}", name=f"qkt{i}")
               for i in range(3)]          # A, B, C
        qkd = [qkp.tile([P, t], BF16, tag=f"qkd{i}", name=f"qkd{i}")
               for i in range(3)]          # A', B', C'
        A, Bt, Ct = qkt
        Ad, Bd, Cd = qkd
        qk0 = [(A, Bt), (Ad, Bd), (Ct, Cd)]      # base-0 (q_tile, k_tile)
        qk64 = [(Ad, Bd), (A, Bt), (Cd, Ct)]     # base-64 (q_tile, k_tile)
        # v in natural orientation: per 128-row t-tile, cols h*65..h*65+64
        # hold [v_h | ones]
        vnat = [vp.tile([P, VW], BF16, tag=f"vn{j}", name=f"vn{j}")
                for j in range(t // P)]

        def v_ap(h, j):
            return vnat[j][:, 65 * h:65 * h + DH + 1]

        def gen_qkv(g):
            """Step-granular generator for group g's QKV filler: one PE
            matmul per step, so the weave can spread them between
            attention matmuls (keeps the PE cadence smooth)."""
            gs = slice(g * QG, (g + 1) * QG)
            if g == 0:
                xtiles[0] = x0_sb           # preloaded with the constants
            else:
                xk = []
                for k in range(6):
                    tl = xpool.tile([P, QG], BF16, tag=f"x{k}", name=f"x{k}")
                    nc.sync.dma_start(tl[:], xT[k * P:(k + 1) * P, gs])
                    xk.append(tl)
                xtiles[g % 2] = xk
            yield
            xk = xtiles[g % 2]
            for m in range(3):              # qkT m-tiles (A, B, C)
                ps = aux.tile([P, QG], F32, tag="aux", name="qkps")
                for k in range(6):
                    nc.tensor.matmul(ps[:], wqk_sb[k][:, m * P:(m + 1) * P],
                                     xk[k][:], start=(k == 0), stop=(k == 5))
                    if k < 5:
                        yield
                nc.vector.tensor_scalar_add(qkt[m][:, gs], ps[:], b_sb[m][:])
                # half-swapped duplicate via SBUF->SBUF DMA (frees ACT)
                nc.sync.dma_start(qkd[m][DH:, gs], qkt[m][:DH, gs])
                nc.sync.dma_start(qkd[m][:DH, gs], qkt[m][DH:, gs])
                yield
            for ti in range(4):             # v t-tiles
                j = 4 * g + ti
                ps = aux.tile([P, NVP], F32, tag="aux", name="vnps")
                for k in range(6):
                    nc.tensor.matmul(ps[:], xk[k][:, ti * P:(ti + 1) * P],
                                     wvp_sb[k][:], start=(k == 0), stop=(k == 5))
                    if k < 5:
                        yield
                nc.vector.tensor_add(vnat[j][:], ps[:, :VW], bv_tile[:, :VW])
                yield

        def gen_proj(g, y0, y1):
            """Step-granular projection for group g."""
            gq = slice(g * QG, (g + 1) * QG)
            for cm in range(6):
                op = aux.tile([P, QG], F32, tag="aux", name="oo")
                nc.tensor.matmul(op[:], wp0[:, cm * P:(cm + 1) * P],
                                 y0[:], start=True, stop=False)
                yield
                nc.tensor.matmul(op[:], wp1[:, cm * P:(cm + 1) * P],
                                 y1[:], start=False, stop=True)
                ob = osb.tile([P, QG], F32, tag="ob", name="ob")
                nc.vector.tensor_copy(ob[:], op[:])
                nc.sync.dma_start(outT[cm * P:(cm + 1) * P, gq], ob[:])
                yield

        def emit_S(g, h, pr, npairs):
            """S^T pair + exp (+mask); returns the pending-PV job."""
            gs = slice(g * QG, (g + 1) * QG)
            q0t, k0t = qk0[h]
            q64t, k64t = qk64[h]
            j0, j1 = 2 * pr, 2 * pr + 1
            s2 = spsum.tile([P, 2 * QG], F32, tag="s", name="s")
            # two k-tiles in disjoint PE row-groups (base 0 / base 64) ->
            # the array runs them concurrently
            nc.tensor.matmul(
                s2[:, :QG],
                k0t[0:DH, j0 * P:(j0 + 1) * P],
                q0t[0:DH, gs],
                start=True, stop=True,
            )
            nc.tensor.matmul(
                s2[:, QG:],
                k64t[DH:P, j1 * P:(j1 + 1) * P],
                q64t[DH:P, gs],
                start=True, stop=True,
            )
            p2 = ppool.tile([P, 2 * QG], BF16, tag="p", name="p")
            nc.scalar.activation(p2[:], s2[:], EXP, scale=float(SCALE))
            if pr >= npairs - 2:
                # causal mask for the 2 diagonal k-tiles, in-place on the
                # Pool engine: keep col q of row k of half j iff
                # (g*512 + q) - (j*128 + k) >= 0.
                nc.gpsimd.affine_select(
                    out=p2[:],
                    in_=p2[:],
                    compare_op=IS_GE,
                    fill=0.0,
                    base=g * QG - j0 * KT,
                    pattern=[[-KT, 2], [1, QG]],
                    channel_multiplier=-1,
                )
            if debug and g == 0 and h == 0 and pr == npairs - 1:
                nc.sync.dma_start(dbg["d_p2"][:, :], p2[:])
            return (g, h, pr, npairs, p2)

        cur_yt = [None]          # live yt PSUM accumulator

        def emit_PV_half(job, half):
            """One PV matmul (k-tile half of a pending job)."""
            g, h, pr, npairs, p2 = job
            first = (pr == 0) and (half == 0)
            last = (pr == npairs - 1) and (half == 1)
            j = 2 * pr + half
            if first:
                cur_yt[0] = ytps.tile([DH + 1, QG], F32, tag="yt",
                                      name="yt")
            yt = cur_yt[0]
            nc.tensor.matmul(yt[:], v_ap(h, j),
                             p2[:, half * QG:(half + 1) * QG],
                             start=first, stop=last)

        def emit_norm(job, yt0, yt1):
            """Head-end normalization: divide rows 0-63 by the
            denominator row (row 64), all off-PE."""
            g, h, pr, npairs, p2 = job
            yt = cur_yt[0]
            # reciprocal_approx_fast requires a partition-base-0 SBUF
            # input on hardware: bounce the denominator row first.
            d_f = rrp.tile([1, QG], F32, tag="df", name="df")
            nc.vector.tensor_copy(d_f[:], yt[DH:DH + 1, :])
            r_f = rrp.tile([1, QG], F32, tag="rf", name="rf")
            nc.vector.reciprocal_approx_fast(r_f[:], d_f[:])
            Rb = rrp.tile([DH, QG], F32, tag="Rb", name="Rb")
            nc.gpsimd.partition_broadcast(Rb[:], r_f[:])
            dest = yt0[DH * h:DH * (h + 1), :] if h < 2 else yt1[:]
            nc.vector.tensor_mul(dest, yt[:DH, :], Rb[:])
            if debug and g == 0 and h == 0:
                nc.sync.dma_start(dbg["d_rb"][:, :], Rb[:])

        xtiles = [None, None]
        # prologue: group 0's QKV back-to-back
        for _ in gen_qkv(0):
            pass

        # ---- fused attention, software-pipelined: PV trails S by DEPTH
        # ---- pair-slots (continuously across heads and groups) so the
        # ---- exp+mask latency never stalls the PE.  QKV of group g+1
        # ---- and proj of group g-1 weave in as single-matmul filler
        # ---- steps between the attention matmuls.
        DEPTH = 2
        NF_QKV = 1 + 3 * 6 + 4 * 6       # steps yielded by gen_qkv
        NF_PROJ = 12                     # steps yielded by gen_proj
        pend = []                # jobs with S emitted, PV outstanding
        yts_of = {}              # group -> (yt0, yt1)
        prev_y = None
        for g in range(ng):
            npairs = 2 * (g + 1)
            yt0 = ytsb.tile([P, QG], BF16, tag="yt0", name="yt0")
            yt1 = ytsb.tile([DH, QG], BF16, tag="yt1", name="yt1")
            yts_of[g] = (yt0, yt1)
            # filler: next group's QKV then prev group's proj
            fill = []            # list of (kind, generator)
            nf_total = 0
            if g + 1 < ng:
                fill.append(("qkv", gen_qkv(g + 1)))
                nf_total += NF_QKV
            if prev_y is not None:
                fill.append(("proj", gen_proj(g - 1, *prev_y)))
                nf_total += NF_PROJ

            def pull_step(g=g, fill=fill, pend=pend):
                """Advance the filler by one step; False when dry or
                blocked (proj before the prev group's norm is out)."""
                while fill:
                    kind, it = fill[0]
                    if kind == "proj" and any(j[0] < g for j in pend):
                        return False
                    try:
                        next(it)
                        return True
                    except StopIteration:
                        fill.pop(0)
                return False

            n_att = npairs * HPC
            next_fill = 0
            ui = 0
            for h in range(HPC):
                for pr in range(npairs):
                    pend.append(emit_S(g, h, pr, npairs))
                    due = (ui + 1) * nf_total // n_att - next_fill
                    jb = pend[0] if len(pend) > DEPTH else None
                    if jb is not None:
                        if due > 0 and pull_step():
                            next_fill += 1
                            due -= 1
                        emit_PV_half(jb, 0)
                        if due > 0 and pull_step():
                            next_fill += 1
                            due -= 1
                        emit_PV_half(jb, 1)
                        if jb[2] == jb[3] - 1:          # pr == npairs-1
                            emit_norm(jb, *yts_of[jb[0]])
                        pend.pop(0)
                    while due > 0 and pull_step():
                        next_fill += 1
                        due -= 1
                    ui += 1
            while pull_step():
                pass
            prev_y = (yt0, yt1)
        # epilogue: drain the pipeline, then the last group's projection
        while pend:
            jb = pend.pop(0)
            emit_PV_half(jb, 0)
            emit_PV_half(jb, 1)
            if jb[2] == jb[3] - 1:
                emit_norm(jb, *yts_of[jb[0]])
        for _ in gen_proj(ng - 1, *prev_y):
            pass
    nc.compile()
    return nc


_NC_CACHE = {}


def get_nc(t=T_FULL):
    if t not in _NC_CACHE:
        _NC_CACHE[t] = build_nc(t)
    return _NC_CACHE[t]


def _bf16(a):
    import ml_dtypes

    return np.ascontiguousarray(np.asarray(a, np.float32)).astype(
        ml_dtypes.bfloat16
    )


def make_in_maps(x, W_attn, b_attn, W_proj):
    x = np.ascontiguousarray(np.asarray(x, np.float32))
    W_attn = np.asarray(W_attn, np.float32)
    b_attn = np.asarray(b_attn, np.float32)
    W_proj = np.asarray(W_proj, np.float32)
    in_maps = []
    for c in range(NCORES):
        b = c // 4
        hs = [3 * (c % 4) + i for i in range(HPC)]
        cols = [W_attn[:, BLK[kind] * C + hs[lh] * DH:
                       BLK[kind] * C + (hs[lh] + 1) * DH]
                for kind, lh in QK_ORDER]
        wqk = np.ascontiguousarray(np.concatenate(cols, axis=1))
        bqk = np.concatenate(
            [b_attn[BLK[kind] * C + hs[lh] * DH:BLK[kind] * C + (hs[lh] + 1) * DH]
             for kind, lh in QK_ORDER]
        ).reshape(NQK, 1)
        wvp = np.zeros((C, NVP), np.float32)
        bvrow = np.zeros((1, NVP), np.float32)
        for lh in range(HPC):
            wvp[:, 65 * lh:65 * lh + DH] = \
                W_attn[:, 2 * C + hs[lh] * DH:2 * C + (hs[lh] + 1) * DH]
            bvrow[0, 65 * lh:65 * lh + DH] = \
                b_attn[2 * C + hs[lh] * DH:2 * C + (hs[lh] + 1) * DH]
            bvrow[0, 65 * lh + DH] = 1.0
        wp = np.ascontiguousarray(
            np.concatenate([W_proj[h * DH:(h + 1) * DH, :] for h in hs], axis=0)
        )
        xTc = np.ascontiguousarray(x[b].T)
        in_maps.append({
            "xT": _bf16(xTc),
            "wqk": _bf16(wqk),
            "bqk": np.ascontiguousarray(bqk),
            "wvp": _bf16(wvp),
            "bvrow": bvrow,
            "wp": _bf16(wp),
        })
    return in_maps


def unshard(per_core_outT, b_proj):
    t = per_core_outT[0].shape[1]
    out = np.zeros((B, t, C), np.float32)
    for c in range(NCORES):
        out[c // 4] += np.asarray(per_core_outT[c], np.float32).T
    out += np.asarray(b_proj, np.float32)[None, None, :]
    return out


def kernel(x, W_attn, b_attn, W_proj, b_proj, **run_kwargs):
    nc = get_nc(T_FULL)
    in_maps = make_in_maps(x, W_attn, b_attn, W_proj)
    res = None
    last_err = None
    for attempt in range(3):
        try:
            res = run_bass_kernel_spmd(nc, in_maps,
                                       core_ids=list(range(NCORES)),
                                       **run_kwargs)
            break
        except Exception as e:  # transient NRT_EXEC_UNIT_UNRECOVERABLE etc.
            last_err = e
    if res is None:
        raise last_err
    outs = [res.results[c]["outT"] for c in range(NCORES)]
    out = unshard(outs, b_proj)
    return out
